# revision 1
# baseline (speedup 1.0000x reference)
"""Fused multi-head attention block (qkv proj + attention + out proj) for
Trainium2, batch-parallel across 8 NeuronCores.

Problem shapes (hardcoded): x [8, 1024, 768], w_qkv [2304, 768],
w_proj [768, 768], b_proj [768]; H=12 heads, HD=64.

Each core processes one batch element b. Layouts:
  qkT  [2C, N]  q,k transposed (bf16): head h -> tile h//2, parts (h%2)*64..
  v_sb [N, H, 65] v natural (bf16) + ones column per head (softmax sums)
  S.T = kT.T @ qT per head, K=64 row-tiled head pairs sharing the PE array
  P.T = exp(S.T/8) on ACT (bf16, max-subtraction skipped: scores ~N(0,1),
        max ~5.5, exp < 300 so fp32 PSUM never overflows)
  [av; sums].T = [V|1].T @ P.T (bf16, M=65), normalized by broadcasting
  1/sums across partitions; attn.T (bf16) -> proj + bias.

Emission interleaves qkv/proj matmul groups into the ACT-paced attention
loop so the PE never idles (keeps HAM at K=8/8).
"""
import numpy as np

import concourse.bacc as bacc
import concourse.tile as tile
from concourse import mybir
from concourse.bass_utils import run_bass_kernel_spmd

B, N, C = 8, 1024, 768
H, HD = 12, 64
P = 128
NCORES = 8
F32 = mybir.dt.float32
F32R = mybir.dt.float32r
BF16 = mybir.dt.bfloat16
Exp = mybir.ActivationFunctionType.Exp

KC = C // P          # 6 contraction chunks of 128 over C
NT = N // P          # 8 npos tiles of 128
QC = 2               # qpos halves of 512
NPAIR = H // 2       # 6 head pairs
SCALE = float(HD) ** -0.5


def build_nc():
    nc = bacc.Bacc("TRN2", target_bir_lowering=False, debug=False)

    xt = nc.declare_dram_parameter("xt", [C, N], F32R, isOutput=False)
    wqk = nc.declare_dram_parameter("wqk", [C, 2 * C], F32R, isOutput=False)
    wv = nc.declare_dram_parameter("wv", [C, C], F32R, isOutput=False)
    wproj = nc.declare_dram_parameter("wproj", [C, C], BF16, isOutput=False)
    bias = nc.declare_dram_parameter("bias", [P, C], F32, isOutput=False)
    out = nc.declare_dram_parameter("out", [N, C], F32, isOutput=True)

    with tile.TileContext(nc) as tc:
        with tc.tile_pool(name="qk", bufs=1) as qk_pool, \
             tc.tile_pool(name="vsb", bufs=1) as v_pool, \
             tc.tile_pool(name="attnT", bufs=1) as at_pool, \
             tc.tile_pool(name="p1in", bufs=1) as p1in, \
             tc.tile_pool(name="p3in", bufs=1) as p3in, \
             tc.tile_pool(name="es", bufs=14) as es_pool, \
             tc.tile_pool(name="rr", bufs=2) as r_pool, \
             tc.tile_pool(name="osb", bufs=3) as o_pool, \
             tc.tile_pool(name="scps", bufs=2, space="PSUM") as sc_ps, \
             tc.tile_pool(name="gps", bufs=4, space="PSUM") as g_ps:

            qk_sb = [qk_pool.tile([P, N], BF16, tag=f"qk{i}", name=f"qk{i}")
                     for i in range(12)]
            v_sb = [v_pool.tile([P, H, 65], BF16, tag=f"v{i}", name=f"v{i}")
                    for i in range(NT)]
            attnT = [at_pool.tile([P, N], BF16, tag=f"at{i}", name=f"at{i}")
                     for i in range(NPAIR)]
            xt_sb = [p1in.tile([P, N], F32R, tag=f"xt{k}", name=f"xts{k}")
                     for k in range(KC)]
            wqk_sb = [p1in.tile([P, 2 * C], F32R, tag=f"wqk{k}", name=f"wqks{k}")
                      for k in range(KC)]
            wv_sb = [p1in.tile([P, C], F32R, tag=f"wv{k}", name=f"wvs{k}")
                     for k in range(KC)]
            wproj_sb = [p3in.tile([P, C], BF16, tag=f"wp{k}", name=f"wps{k}")
                        for k in range(KC)]
            bias_sb = p3in.tile([P, C], F32, tag="bias", name="biassb")

            # DMAs: xt + the wqk column slices used first (mt 0,1 / 6,7),
            # then the rest; weights for later phases last.
            for k in range(KC):
                nc.sync.dma_start(out=xt_sb[k][:, 0:512],
                                  in_=xt[k * P:(k + 1) * P, 0:512])
                nc.sync.dma_start(out=wqk_sb[k][:, 0:256],
                                  in_=wqk[k * P:(k + 1) * P, 0:256])
                nc.sync.dma_start(out=wqk_sb[k][:, 768:1024],
                                  in_=wqk[k * P:(k + 1) * P, 768:1024])
            for k in range(KC):
                nc.scalar.dma_start(out=wv_sb[k][:], in_=wv[k * P:(k + 1) * P, :])
            for k in range(KC):
                nc.scalar.dma_start(out=xt_sb[k][:, 512:1024],
                                  in_=xt[k * P:(k + 1) * P, 512:1024])
            for k in range(KC):
                nc.sync.dma_start(out=wqk_sb[k][:, 256:768],
                                  in_=wqk[k * P:(k + 1) * P, 256:768])
                nc.sync.dma_start(out=wqk_sb[k][:, 1024:1536],
                                  in_=wqk[k * P:(k + 1) * P, 1024:1536])
            for k in range(KC):
                nc.scalar.dma_start(out=wproj_sb[k][:], in_=wproj[k * P:(k + 1) * P, :])
            nc.scalar.dma_start(out=bias_sb[:], in_=bias[:, :])

            def emit_qkT(mt, nh):
                ps = g_ps.tile([P, 512], F32, tag="g", name="gq")
                for k in range(KC):
                    nc.tensor.matmul(
                        ps[:],
                        wqk_sb[k][:, mt * P:(mt + 1) * P],
                        xt_sb[k][:, nh * 512:(nh + 1) * 512],
                        start=(k == 0), stop=(k == KC - 1),
                    )
                nc.vector.tensor_copy(qk_sb[mt][:, nh * 512:(nh + 1) * 512], ps[:])

            def emit_v(nt, ci):
                c0, cw = ((0, 512), (512, 256))[ci]
                ps = g_ps.tile([P, 512], F32, tag="g", name="gv")
                for k in range(KC):
                    nc.tensor.matmul(
                        ps[:, :cw],
                        xt_sb[k][:, nt * P:(nt + 1) * P],
                        wv_sb[k][:, c0:c0 + cw],
                        start=(k == 0), stop=(k == KC - 1),
                    )
                psv = ps[:, :cw].rearrange("p (j q) -> p j q", q=64)
                nc.vector.tensor_copy(
                    v_sb[nt][:, c0 // 64:c0 // 64 + cw // 64, 0:64], psv[:])

            def av_alloc():
                return [g_ps.tile([P, 512], F32, tag="g", name="gav")
                        for _ in range(2)]

            def av_mms(p, es_tiles, av_ps2, kt):
                for par in range(2):
                    nc.tensor.matmul(
                        av_ps2[par][0:65, :],
                        v_sb[kt][:, 2 * p + par, :],
                        es_tiles[kt][:, par * 512:(par + 1) * 512],
                        start=(kt == 0), stop=(kt == NT - 1),
                    )

            def av_norm(p, qc, av_ps2):
                for par in range(2):
                    av = av_ps2[par]
                    # evict PSUM right away so the psum slot frees without
                    # waiting on the normalization chain
                    av_sb = r_pool.tile([P, 512], F32, tag="avsb", name="avsb")
                    nc.vector.tensor_copy(av_sb[0:65, :], av[0:65, :])
                    # stock DVE op: part 64 -> part 0 (cross-quadrant ok)
                    rrow = r_pool.tile([P, 512], F32, tag="rrow", name="rrow")
                    nc.vector.tensor_copy(rrow[0:1, :], av_sb[64:65, :])
                    sbc = r_pool.tile([P, 512], F32, tag="sbc", name="sbc")
                    nc.gpsimd.partition_broadcast(sbc[0:64, :], rrow[0:1, :])
                    rbc = r_pool.tile([P, 512], F32, tag="rbc", name="rbc")
                    # custom-DVE op: base partition 0 only
                    nc.vector.reciprocal_approx_fast(rbc[0:64, :], sbc[0:64, :])
                    # 64-channel DVE op writes the head's attnT quadrant
                    nc.vector.tensor_mul(
                        attnT[p][par * 64:(par + 1) * 64, qc * 512:(qc + 1) * 512],
                        av_sb[0:64, :],
                        rbc[0:64, :])

            proj_osb = {}

            def emit_proj(nt, ci):
                c0, cw = ((0, 512), (512, 256))[ci]
                ps = g_ps.tile([P, 512], F32, tag="g", name="gp")
                for k in range(KC):
                    nc.tensor.matmul(
                        ps[:, :cw],
                        attnT[k][:, nt * P:(nt + 1) * P],
                        wproj_sb[k][:, c0:c0 + cw],
                        start=(k == 0), stop=(k == KC - 1),
                    )
                if ci == 0:
                    proj_osb[nt] = o_pool.tile([P, C], F32, tag="o", name="osb")
                o_sb = proj_osb[nt]
                nc.vector.tensor_add(o_sb[:, c0:c0 + cw], ps[:, :cw],
                                     bias_sb[:, c0:c0 + cw])
                nc.sync.dma_start(out=out[nt * P:(nt + 1) * P, c0:c0 + cw],
                                  in_=o_sb[:, c0:c0 + cw])

            def emit_scores_kt(p, qc, kt):
                ps = sc_ps.tile([P, N], F32, tag="sc", name="scps")
                nc.tensor.matmul(
                    ps[:, 0:512],
                    qk_sb[6 + p][0:64, kt * P:(kt + 1) * P],
                    qk_sb[p][0:64, qc * 512:(qc + 1) * 512],
                    start=True, stop=True, tile_position=(0, 0),
                )
                nc.tensor.matmul(
                    ps[:, 512:1024],
                    qk_sb[6 + p][64:128, kt * P:(kt + 1) * P],
                    qk_sb[p][64:128, qc * 512:(qc + 1) * 512],
                    start=True, stop=True, tile_position=(64, 0),
                )
                es = es_pool.tile([P, N], BF16, tag="es", name="es")
                nc.scalar.activation(es[:], ps[:], Exp, scale=SCALE)
                return es

            # ---------- PRE: qkT for pair 0 + all of v ----------
            for nt in range(NT):
                nc.vector.memset(v_sb[nt][:, :, 64:65], 1.0)
            emit_qkT(0, 0)
            emit_qkT(6, 0)
            for nt in range(4):
                emit_v(nt, 0)
                emit_v(nt, 1)
            emit_qkT(0, 1)
            emit_qkT(6, 1)
            for nt in range(4, NT):
                emit_v(nt, 0)
                emit_v(nt, 1)

            # ---------- attention with interleaved fillers ----------
            # iters 0..4 fillers: remaining qkT M-tiles (one pair ahead of
            # the scores that consume them); iters 6..9: proj of qc0 rows
            filler_map = {
                0: [(emit_qkT, (1, 0)), (emit_qkT, (1, 1)),
                    (emit_qkT, (7, 0)), (emit_qkT, (7, 1))],
                1: [(emit_qkT, (2, 0)), (emit_qkT, (2, 1)),
                    (emit_qkT, (8, 0)), (emit_qkT, (8, 1))],
                2: [(emit_qkT, (3, 0)), (emit_qkT, (3, 1)),
                    (emit_qkT, (9, 0)), (emit_qkT, (9, 1))],
                3: [(emit_qkT, (4, 0)), (emit_qkT, (4, 1)),
                    (emit_qkT, (10, 0)), (emit_qkT, (10, 1))],
                4: [(emit_qkT, (5, 0)), (emit_qkT, (5, 1)),
                    (emit_qkT, (11, 0)), (emit_qkT, (11, 1))],
                7: [(emit_proj, (0, 0)), (emit_proj, (0, 1))],
                8: [(emit_proj, (1, 0)), (emit_proj, (1, 1))],
                9: [(emit_proj, (2, 0)), (emit_proj, (2, 1))],
                10: [(emit_proj, (3, 0)), (emit_proj, (3, 1))],
            }
            pending = None
            for it in range(12):
                qc, p = it // 6, it % 6
                fillers = list(filler_map.get(it, []))
                av_ps2 = av_alloc() if pending is not None else None
                es_tiles = []
                for kt in range(NT):
                    es_tiles.append(emit_scores_kt(p, qc, kt))
                    if pending is not None:
                        # interleave previous pair's av accumulation between
                        # scores pairs (fills PE while exp runs, and lets the
                        # scores LDWEIGHTS background-load without row-group
                        # conflicts)
                        av_mms(pending[0], pending[2], av_ps2, kt)
                    if kt % 2 == 1 and fillers:
                        fn, args = fillers.pop(0)
                        fn(*args)
                for fn, args in fillers:
                    fn(*args)
                if pending is not None:
                    av_norm(pending[0], pending[1], av_ps2)
                pending = (p, qc, es_tiles)
            av_ps2 = av_alloc()
            for kt in range(NT):
                av_mms(pending[0], pending[2], av_ps2, kt)
            av_norm(pending[0], pending[1], av_ps2)
            # tail: proj of qc1 rows
            for nt in range(4, NT):
                emit_proj(nt, 0)
                emit_proj(nt, 1)

    nc.finalize()
    return nc


_NC_CACHE = None


def _get_nc():
    global _NC_CACHE
    if _NC_CACHE is None:
        _NC_CACHE = build_nc()
    return _NC_CACHE


def prep_inputs(x, w_qkv, w_proj, b_proj):
    import ml_dtypes
    x = np.asarray(x, dtype=np.float32)
    w_qkv = np.asarray(w_qkv, dtype=np.float32)
    w_proj = np.asarray(w_proj, dtype=np.float32)
    b_proj = np.asarray(b_proj, dtype=np.float32)
    wqk = np.ascontiguousarray(w_qkv[:2 * C].T)          # [768, 1536]
    wv = np.ascontiguousarray(w_qkv[2 * C:].T)           # [768, 768]
    wp = np.ascontiguousarray(w_proj.T).astype(ml_dtypes.bfloat16)
    bias = np.ascontiguousarray(np.tile(b_proj[None, :], (P, 1)))  # [128, 768]
    in_maps = []
    for b in range(NCORES):
        in_maps.append({
            "xt": np.ascontiguousarray(x[b].T),          # [768, 1024]
            "wqk": wqk, "wv": wv, "wproj": wp, "bias": bias,
        })
    return in_maps


def run(in_maps, **kw):
    nc = _get_nc()
    return run_bass_kernel_spmd(nc, in_maps, list(range(NCORES)), **kw)


def kernel(x, w_qkv, w_proj, b_proj):
    res = run(prep_inputs(x, w_qkv, w_proj, b_proj))
    return np.stack([res.results[b]["out"] for b in range(NCORES)], axis=0)



# revision 2
# speedup vs baseline: 1.0401x; 1.0401x over previous
"""Fused multi-head attention block (qkv proj + attention + out proj) for
Trainium2, batch-parallel across 8 NeuronCores.

Problem shapes (hardcoded): x [8, 1024, 768], w_qkv [2304, 768],
w_proj [768, 768], b_proj [768]; H=12 heads, HD=64.

Each core processes one batch element b. Layouts:
  qkT  [2C, N]  q,k transposed (bf16): head h -> tile h//2, parts (h%2)*64..
  v_sb [N, H, 65] v natural (bf16) + ones column per head (softmax sums)
  S.T = kT.T @ qT per head, K=64 row-tiled head pairs sharing the PE array
  P.T = exp(S.T/8) on ACT (bf16, max-subtraction skipped: scores ~N(0,1),
        max ~5.5, exp < 300 so fp32 PSUM never overflows)
  [av; sums].T = [V|1].T @ P.T (bf16, M=65), normalized by broadcasting
  1/sums across partitions; attn.T (bf16) -> proj + bias.

x / w_qkv / w_v stream in as bf16 (halves input DMA; rel err ~8e-3 vs
2e-2 budget). DMAs are ordered to match emission order so the PE never
waits at startup. Emission interleaves qkv/proj matmul groups into the
ACT-paced attention loop so the PE never idles; the final pair's AV is
interleaved into its own scores iteration and the tail projections are
split into k0-4 (independent of the last softmax norm) and k5
(dependent) so the tail has no serial PE stall.
"""
import numpy as np

import concourse.bacc as bacc
import concourse.tile as tile
from concourse import mybir
from concourse.bass_utils import run_bass_kernel_spmd

B, N, C = 8, 1024, 768
H, HD = 12, 64
P = 128
NCORES = 8
F32 = mybir.dt.float32
BF16 = mybir.dt.bfloat16
Exp = mybir.ActivationFunctionType.Exp

KC = C // P          # 6 contraction chunks of 128 over C
NT = N // P          # 8 npos tiles of 128
QC = 2               # qpos halves of 512
NPAIR = H // 2       # 6 head pairs
SCALE = float(HD) ** -0.5


def build_nc():
    nc = bacc.Bacc("TRN2", target_bir_lowering=False, debug=False)

    xt = nc.declare_dram_parameter("xt", [C, N], BF16, isOutput=False)
    wqk = nc.declare_dram_parameter("wqk", [C, 2 * C], BF16, isOutput=False)
    wv = nc.declare_dram_parameter("wv", [C, C], BF16, isOutput=False)
    wproj = nc.declare_dram_parameter("wproj", [C, C], BF16, isOutput=False)
    bias = nc.declare_dram_parameter("bias", [P, C], F32, isOutput=False)
    out = nc.declare_dram_parameter("out", [N, C], F32, isOutput=True)

    with tile.TileContext(nc) as tc:
        with tc.tile_pool(name="qk", bufs=1) as qk_pool, \
             tc.tile_pool(name="vsb", bufs=1) as v_pool, \
             tc.tile_pool(name="attnT", bufs=1) as at_pool, \
             tc.tile_pool(name="p1in", bufs=1) as p1in, \
             tc.tile_pool(name="p3in", bufs=1) as p3in, \
             tc.tile_pool(name="es", bufs=16) as es_pool, \
             tc.tile_pool(name="rr", bufs=2) as r_pool, \
             tc.tile_pool(name="osb", bufs=3) as o_pool, \
             tc.tile_pool(name="scps", bufs=2, space="PSUM") as sc_ps, \
             tc.tile_pool(name="gps", bufs=4, space="PSUM") as g_ps:

            qk_sb = [qk_pool.tile([P, N], BF16, tag=f"qk{i}", name=f"qk{i}")
                     for i in range(12)]
            v_sb = [v_pool.tile([P, H, 65], BF16, tag=f"v{i}", name=f"v{i}")
                    for i in range(NT)]
            attnT = [at_pool.tile([P, N], BF16, tag=f"at{i}", name=f"at{i}")
                     for i in range(NPAIR)]
            xt_sb = [p1in.tile([P, N], BF16, tag=f"xt{k}", name=f"xts{k}")
                     for k in range(KC)]
            wqk_sb = [p1in.tile([P, 2 * C], BF16, tag=f"wqk{k}", name=f"wqks{k}")
                      for k in range(KC)]
            wv_sb = [p1in.tile([P, C], BF16, tag=f"wv{k}", name=f"wvs{k}")
                     for k in range(KC)]
            wproj_sb = [p3in.tile([P, C], BF16, tag=f"wp{k}", name=f"wps{k}")
                        for k in range(KC)]
            bias_sb = p3in.tile([P, C], F32, tag="bias", name="biassb")

            # DMAs ordered to match PE emission: v tiles (xt cols 0:512 +
            # wv) feed the first emits, then the qkT slices for pair 0,
            # the second xt half, the remaining wqk columns, and the
            # proj-phase weights last. Two queues (sync/scalar) so xt/wqk
            # and wv/wproj stream concurrently.
            for k in range(KC):
                nc.sync.dma_start(out=xt_sb[k][:, 0:512],
                                  in_=xt[k * P:(k + 1) * P, 0:512])
            for k in range(KC):
                nc.scalar.dma_start(out=wv_sb[k][:, 0:512],
                                    in_=wv[k * P:(k + 1) * P, 0:512])
            for k in range(KC):
                nc.sync.dma_start(out=wqk_sb[k][:, 0:128],
                                  in_=wqk[k * P:(k + 1) * P, 0:128])
                nc.sync.dma_start(out=wqk_sb[k][:, 768:896],
                                  in_=wqk[k * P:(k + 1) * P, 768:896])
            for k in range(KC):
                nc.scalar.dma_start(out=wv_sb[k][:, 512:768],
                                    in_=wv[k * P:(k + 1) * P, 512:768])
            for k in range(KC):
                nc.sync.dma_start(out=xt_sb[k][:, 512:1024],
                                  in_=xt[k * P:(k + 1) * P, 512:1024])
            for k in range(KC):
                nc.sync.dma_start(out=wqk_sb[k][:, 128:768],
                                  in_=wqk[k * P:(k + 1) * P, 128:768])
                nc.sync.dma_start(out=wqk_sb[k][:, 896:1536],
                                  in_=wqk[k * P:(k + 1) * P, 896:1536])
            for k in range(KC):
                nc.scalar.dma_start(out=wproj_sb[k][:], in_=wproj[k * P:(k + 1) * P, :])
            nc.scalar.dma_start(out=bias_sb[:], in_=bias[:, :])

            def emit_qkT(mt, nh):
                ps = g_ps.tile([P, 512], F32, tag="g", name="gq")
                for k in range(KC):
                    nc.tensor.matmul(
                        ps[:],
                        wqk_sb[k][:, mt * P:(mt + 1) * P],
                        xt_sb[k][:, nh * 512:(nh + 1) * 512],
                        start=(k == 0), stop=(k == KC - 1),
                    )
                nc.vector.tensor_copy(qk_sb[mt][:, nh * 512:(nh + 1) * 512], ps[:])

            def emit_v(nt, ci):
                c0, cw = ((0, 512), (512, 256))[ci]
                ps = g_ps.tile([P, 512], F32, tag="g", name="gv")
                for k in range(KC):
                    nc.tensor.matmul(
                        ps[:, :cw],
                        xt_sb[k][:, nt * P:(nt + 1) * P],
                        wv_sb[k][:, c0:c0 + cw],
                        start=(k == 0), stop=(k == KC - 1),
                    )
                psv = ps[:, :cw].rearrange("p (j q) -> p j q", q=64)
                nc.vector.tensor_copy(
                    v_sb[nt][:, c0 // 64:c0 // 64 + cw // 64, 0:64], psv[:])

            def av_alloc():
                return [g_ps.tile([P, 512], F32, tag="g", name="gav")
                        for _ in range(2)]

            def av_mms(p, es_tiles, av_ps2, kt):
                for par in range(2):
                    nc.tensor.matmul(
                        av_ps2[par][0:65, :],
                        v_sb[kt][:, 2 * p + par, :],
                        es_tiles[kt][:, par * 512:(par + 1) * 512],
                        start=(kt == 0), stop=(kt == NT - 1),
                    )

            def av_norm(p, qc, av_ps2):
                for par in range(2):
                    av = av_ps2[par]
                    # evict PSUM right away so the psum slot frees without
                    # waiting on the normalization chain
                    av_sb = r_pool.tile([P, 512], F32, tag="avsb", name="avsb")
                    nc.vector.tensor_copy(av_sb[0:65, :], av[0:65, :])
                    # stock DVE op: part 64 -> part 0 (cross-quadrant ok)
                    rrow = r_pool.tile([P, 512], F32, tag="rrow", name="rrow")
                    nc.vector.tensor_copy(rrow[0:1, :], av_sb[64:65, :])
                    sbc = r_pool.tile([P, 512], F32, tag="sbc", name="sbc")
                    nc.gpsimd.partition_broadcast(sbc[0:64, :], rrow[0:1, :])
                    rbc = r_pool.tile([P, 512], F32, tag="rbc", name="rbc")
                    # custom-DVE op: base partition 0 only
                    nc.vector.reciprocal_approx_fast(rbc[0:64, :], sbc[0:64, :])
                    # 64-channel DVE op writes the head's attnT quadrant
                    nc.vector.tensor_mul(
                        attnT[p][par * 64:(par + 1) * 64, qc * 512:(qc + 1) * 512],
                        av_sb[0:64, :],
                        rbc[0:64, :])

            proj_osb = {}
            proj_ps = {}

            def proj_mms(nt, ci, ks, ke):
                c0, cw = ((0, 512), (512, 256))[ci]
                if ks == 0:
                    proj_ps[(nt, ci)] = g_ps.tile([P, 512], F32, tag="g",
                                                  name="gp")
                ps = proj_ps[(nt, ci)]
                for k in range(ks, ke):
                    nc.tensor.matmul(
                        ps[:, :cw],
                        attnT[k][:, nt * P:(nt + 1) * P],
                        wproj_sb[k][:, c0:c0 + cw],
                        start=(k == 0), stop=(k == KC - 1),
                    )

            def proj_fin(nt, ci):
                c0, cw = ((0, 512), (512, 256))[ci]
                ps = proj_ps.pop((nt, ci))
                if ci == 0:
                    proj_osb[nt] = o_pool.tile([P, C], F32, tag="o", name="osb")
                o_sb = proj_osb[nt]
                nc.vector.tensor_add(o_sb[:, c0:c0 + cw], ps[:, :cw],
                                     bias_sb[:, c0:c0 + cw])
                nc.sync.dma_start(out=out[nt * P:(nt + 1) * P, c0:c0 + cw],
                                  in_=o_sb[:, c0:c0 + cw])

            def emit_proj(nt, ci):
                proj_mms(nt, ci, 0, KC)
                proj_fin(nt, ci)

            def emit_scores_kt(p, qc, kt):
                ps = sc_ps.tile([P, N], F32, tag="sc", name="scps")
                nc.tensor.matmul(
                    ps[:, 0:512],
                    qk_sb[6 + p][0:64, kt * P:(kt + 1) * P],
                    qk_sb[p][0:64, qc * 512:(qc + 1) * 512],
                    start=True, stop=True, tile_position=(0, 0),
                )
                nc.tensor.matmul(
                    ps[:, 512:1024],
                    qk_sb[6 + p][64:128, kt * P:(kt + 1) * P],
                    qk_sb[p][64:128, qc * 512:(qc + 1) * 512],
                    start=True, stop=True, tile_position=(64, 0),
                )
                es = es_pool.tile([P, N], BF16, tag="es", name="es")
                nc.scalar.activation(es[:], ps[:], Exp, scale=SCALE)
                return es

            # ---------- PRE: v + qkT for pair 0, in DMA-arrival order ----
            for nt in range(NT):
                nc.vector.memset(v_sb[nt][:, :, 64:65], 1.0)
            for nt in range(4):
                emit_v(nt, 0)
            emit_qkT(0, 0)
            emit_qkT(6, 0)
            for nt in range(4):
                emit_v(nt, 1)
            for nt in range(4, NT):
                emit_v(nt, 0)
                emit_v(nt, 1)
            emit_qkT(0, 1)
            emit_qkT(6, 1)

            # ---------- attention with interleaved fillers ----------
            # iters 0..4 fillers: remaining qkT M-tiles (one pair ahead of
            # the scores that consume them); iters 6..9: proj of qc0 rows
            filler_map = {
                0: [(emit_qkT, (1, 0)), (emit_qkT, (1, 1)),
                    (emit_qkT, (7, 0)), (emit_qkT, (7, 1))],
                1: [(emit_qkT, (2, 0)), (emit_qkT, (2, 1)),
                    (emit_qkT, (8, 0)), (emit_qkT, (8, 1))],
                2: [(emit_qkT, (3, 0)), (emit_qkT, (3, 1)),
                    (emit_qkT, (9, 0)), (emit_qkT, (9, 1))],
                3: [(emit_qkT, (4, 0)), (emit_qkT, (4, 1)),
                    (emit_qkT, (10, 0)), (emit_qkT, (10, 1))],
                4: [(emit_qkT, (5, 0)), (emit_qkT, (5, 1)),
                    (emit_qkT, (11, 0)), (emit_qkT, (11, 1))],
                7: [(emit_proj, (0, 0)), (emit_proj, (0, 1))],
                8: [(emit_proj, (1, 0)), (emit_proj, (1, 1))],
                9: [(emit_proj, (2, 0)), (emit_proj, (2, 1))],
                10: [(emit_proj, (3, 0)), (emit_proj, (3, 1))],
            }
            pending = None
            self_av = None
            for it in range(12):
                qc, p = it // 6, it % 6
                fillers = list(filler_map.get(it, []))
                av_ps2 = av_alloc() if pending is not None else None
                # last iteration also drains its own AV (lag 2 behind the
                # exp pipeline) so the tail has no standalone AV pass
                if it == 11:
                    self_av = av_alloc()
                es_tiles = []
                for kt in range(NT):
                    es_tiles.append(emit_scores_kt(p, qc, kt))
                    if pending is not None:
                        # interleave previous pair's av accumulation between
                        # scores pairs (fills PE while exp runs, and lets the
                        # scores LDWEIGHTS background-load without row-group
                        # conflicts)
                        av_mms(pending[0], pending[2], av_ps2, kt)
                    if self_av is not None and kt >= 2:
                        av_mms(p, es_tiles, self_av, kt - 2)
                    if kt % 2 == 1 and fillers:
                        fn, args = fillers.pop(0)
                        fn(*args)
                for fn, args in fillers:
                    fn(*args)
                if pending is not None:
                    av_norm(pending[0], pending[1], av_ps2)
                pending = (p, qc, es_tiles)

            # ---------- tail: finish (5,1) av + norm, overlap with the ----
            # qc1 projections (k0-4 are independent of the last norm)
            es_tiles = pending[2]
            proj_mms(4, 0, 0, KC - 1)
            proj_mms(4, 1, 0, KC - 1)
            av_mms(5, es_tiles, self_av, 6)
            av_mms(5, es_tiles, self_av, 7)
            av_norm(5, 1, self_av)
            proj_mms(5, 0, 0, KC - 1)
            proj_mms(5, 1, 0, KC - 1)
            proj_mms(4, 0, KC - 1, KC)
            proj_fin(4, 0)
            proj_mms(4, 1, KC - 1, KC)
            proj_fin(4, 1)
            proj_mms(5, 0, KC - 1, KC)
            proj_fin(5, 0)
            proj_mms(5, 1, KC - 1, KC)
            proj_fin(5, 1)
            for nt in range(6, NT):
                emit_proj(nt, 0)
                emit_proj(nt, 1)

    nc.finalize()
    return nc


_NC_CACHE = None


def _get_nc():
    global _NC_CACHE
    if _NC_CACHE is None:
        _NC_CACHE = build_nc()
    return _NC_CACHE


def prep_inputs(x, w_qkv, w_proj, b_proj):
    import ml_dtypes
    x = np.asarray(x, dtype=np.float32)
    w_qkv = np.asarray(w_qkv, dtype=np.float32)
    w_proj = np.asarray(w_proj, dtype=np.float32)
    b_proj = np.asarray(b_proj, dtype=np.float32)
    bf16 = ml_dtypes.bfloat16
    wqk = np.ascontiguousarray(w_qkv[:2 * C].T).astype(bf16)   # [768, 1536]
    wv = np.ascontiguousarray(w_qkv[2 * C:].T).astype(bf16)    # [768, 768]
    wp = np.ascontiguousarray(w_proj.T).astype(bf16)
    bias = np.ascontiguousarray(np.tile(b_proj[None, :], (P, 1)))  # [128, 768]
    in_maps = []
    for b in range(NCORES):
        in_maps.append({
            "xt": np.ascontiguousarray(x[b].T).astype(bf16),   # [768, 1024]
            "wqk": wqk, "wv": wv, "wproj": wp, "bias": bias,
        })
    return in_maps


def run(in_maps, **kw):
    nc = _get_nc()
    return run_bass_kernel_spmd(nc, in_maps, list(range(NCORES)), **kw)


def kernel(x, w_qkv, w_proj, b_proj):
    res = run(prep_inputs(x, w_qkv, w_proj, b_proj))
    return np.stack([res.results[b]["out"] for b in range(NCORES)], axis=0)


# revision 6
# speedup vs baseline: 1.0514x; 1.0109x over previous
"""Fused multi-head attention block (qkv proj + attention + out proj) for
Trainium2, batch-parallel across 8 NeuronCores.

Problem shapes (hardcoded): x [8, 1024, 768], w_qkv [2304, 768],
w_proj [768, 768], b_proj [768]; H=12 heads, HD=64.

Each core processes one batch element b. Layouts:
  qkT  [2C, N]  q,k transposed (bf16): head h -> tile h//2, parts (h%2)*64..
  v_sb [N, H, 65] v natural (bf16) + ones column per head (softmax sums)
  S.T = kT.T @ qT per head, K=64 row-tiled head pairs sharing the PE array
  P.T = exp(S.T/8) on ACT (bf16, max-subtraction skipped: scores ~N(0,1),
        max ~5.5, exp < 300 so fp32 PSUM never overflows)
  [av; sums].T = [V|1].T @ P.T (bf16, M=65), normalized by broadcasting
  1/sums across partitions; attn.T (bf16) -> proj + bias.

x / w_qkv / w_v stream in as bf16 (halves input DMA; rel err ~8e-3 vs
2e-2 budget). DMAs are ordered to match emission order so the PE never
waits at startup. Emission interleaves qkv/proj matmul groups into the
ACT-paced attention loop so the PE never idles; the final pair's AV is
interleaved into its own scores iteration and the tail projections are
split into k0-4 (independent of the last softmax norm) and k5
(dependent) so the tail has no serial PE stall.
"""
import numpy as np

import concourse.bacc as bacc
import concourse.tile as tile
from concourse import mybir
from concourse.bass_utils import run_bass_kernel_spmd

B, N, C = 8, 1024, 768
H, HD = 12, 64
P = 128
NCORES = 8
F32 = mybir.dt.float32
BF16 = mybir.dt.bfloat16
Exp = mybir.ActivationFunctionType.Exp

KC = C // P          # 6 contraction chunks of 128 over C
NT = N // P          # 8 npos tiles of 128
QC = 2               # qpos halves of 512
NPAIR = H // 2       # 6 head pairs
SCALE = float(HD) ** -0.5


def build_nc():
    nc = bacc.Bacc("TRN2", target_bir_lowering=False, debug=False)

    xt = nc.declare_dram_parameter("xt", [C, N], BF16, isOutput=False)
    wqk = nc.declare_dram_parameter("wqk", [C, 2 * C], BF16, isOutput=False)
    wv = nc.declare_dram_parameter("wv", [C, C], BF16, isOutput=False)
    wproj = nc.declare_dram_parameter("wproj", [C, C], BF16, isOutput=False)
    bias = nc.declare_dram_parameter("bias", [P, C], F32, isOutput=False)
    out = nc.declare_dram_parameter("out", [N, C], F32, isOutput=True)

    with tile.TileContext(nc) as tc:
        with tc.tile_pool(name="qk", bufs=1) as qk_pool, \
             tc.tile_pool(name="vsb", bufs=1) as v_pool, \
             tc.tile_pool(name="attnT", bufs=1) as at_pool, \
             tc.tile_pool(name="p1in", bufs=1) as p1in, \
             tc.tile_pool(name="p3in", bufs=1) as p3in, \
             tc.tile_pool(name="es", bufs=16) as es_pool, \
             tc.tile_pool(name="rr", bufs=2) as r_pool, \
             tc.tile_pool(name="osb", bufs=3) as o_pool, \
             tc.tile_pool(name="scps", bufs=2, space="PSUM") as sc_ps, \
             tc.tile_pool(name="gps", bufs=4, space="PSUM") as g_ps:

            qk_sb = [qk_pool.tile([P, N], BF16, tag=f"qk{i}", name=f"qk{i}")
                     for i in range(12)]
            v_sb = [v_pool.tile([P, H, 65], BF16, tag=f"v{i}", name=f"v{i}")
                    for i in range(NT)]
            attnT = [at_pool.tile([P, N], BF16, tag=f"at{i}", name=f"at{i}")
                     for i in range(NPAIR)]
            xt_sb = [p1in.tile([P, N], BF16, tag=f"xt{k}", name=f"xts{k}")
                     for k in range(KC)]
            wqk_sb = [p1in.tile([P, 2 * C], BF16, tag=f"wqk{k}", name=f"wqks{k}")
                      for k in range(KC)]
            wv_sb = [p1in.tile([P, C], BF16, tag=f"wv{k}", name=f"wvs{k}")
                     for k in range(KC)]
            wproj_sb = [p3in.tile([P, C], BF16, tag=f"wp{k}", name=f"wps{k}")
                        for k in range(KC)]
            bias_sb = p3in.tile([P, C], F32, tag="bias", name="biassb")

            # DMAs ordered to match PE emission: v tiles (xt cols 0:512 +
            # wv) feed the first emits, then the qkT slices for pair 0,
            # the second xt half, the remaining wqk columns, and the
            # proj-phase weights last. Two queues (sync/scalar) so xt/wqk
            # and wv/wproj stream concurrently.
            for k in range(KC):
                nc.sync.dma_start(out=xt_sb[k][:, 0:512],
                                  in_=xt[k * P:(k + 1) * P, 0:512])
            for k in range(KC):
                nc.scalar.dma_start(out=wv_sb[k][:, 0:512],
                                    in_=wv[k * P:(k + 1) * P, 0:512])
            for k in range(KC):
                nc.sync.dma_start(out=wqk_sb[k][:, 0:128],
                                  in_=wqk[k * P:(k + 1) * P, 0:128])
                nc.sync.dma_start(out=wqk_sb[k][:, 768:896],
                                  in_=wqk[k * P:(k + 1) * P, 768:896])
            for k in range(KC):
                nc.scalar.dma_start(out=wv_sb[k][:, 512:768],
                                    in_=wv[k * P:(k + 1) * P, 512:768])
            for k in range(KC):
                nc.sync.dma_start(out=xt_sb[k][:, 512:1024],
                                  in_=xt[k * P:(k + 1) * P, 512:1024])
            for k in range(KC):
                nc.sync.dma_start(out=wqk_sb[k][:, 128:768],
                                  in_=wqk[k * P:(k + 1) * P, 128:768])
                nc.sync.dma_start(out=wqk_sb[k][:, 896:1536],
                                  in_=wqk[k * P:(k + 1) * P, 896:1536])
            for k in range(KC):
                nc.scalar.dma_start(out=wproj_sb[k][:], in_=wproj[k * P:(k + 1) * P, :])
            nc.scalar.dma_start(out=bias_sb[:], in_=bias[:, :])

            def emit_qkT(mt, nh):
                ps = g_ps.tile([P, 512], F32, tag="g", name="gq")
                for k in range(KC):
                    nc.tensor.matmul(
                        ps[:],
                        wqk_sb[k][:, mt * P:(mt + 1) * P],
                        xt_sb[k][:, nh * 512:(nh + 1) * 512],
                        start=(k == 0), stop=(k == KC - 1),
                    )
                nc.vector.tensor_copy(qk_sb[mt][:, nh * 512:(nh + 1) * 512], ps[:])

            def emit_v(nt, ci):
                c0, cw = ((0, 512), (512, 256))[ci]
                ps = g_ps.tile([P, 512], F32, tag="g", name="gv")
                for k in range(KC):
                    nc.tensor.matmul(
                        ps[:, :cw],
                        xt_sb[k][:, nt * P:(nt + 1) * P],
                        wv_sb[k][:, c0:c0 + cw],
                        start=(k == 0), stop=(k == KC - 1),
                    )
                psv = ps[:, :cw].rearrange("p (j q) -> p j q", q=64)
                nc.vector.tensor_copy(
                    v_sb[nt][:, c0 // 64:c0 // 64 + cw // 64, 0:64], psv[:])

            def av_alloc():
                return [g_ps.tile([P, 512], F32, tag="g", name="gav")
                        for _ in range(2)]

            def av_mms(p, es_tiles, av_ps2, kt):
                for par in range(2):
                    nc.tensor.matmul(
                        av_ps2[par][0:65, :],
                        v_sb[kt][:, 2 * p + par, :],
                        es_tiles[kt][:, par * 512:(par + 1) * 512],
                        start=(kt == 0), stop=(kt == NT - 1),
                    )

            def av_norm(p, qc, av_ps2, halves=1):
                # reciprocal runs on the [1,512] sums row BEFORE the
                # broadcast (bc(recip(x)) == recip(bc(x))), and the sums
                # row is read straight out of PSUM in parallel with the
                # data eviction -- short serial chain, less DVE work.
                # halves=2 splits bc+mul into q-halves so downstream proj
                # k5 consumers of the first half unblock earlier.
                avsb2, rbc2 = [], []
                for par in range(2):
                    av = av_ps2[par]
                    rrow = r_pool.tile([P, 512], F32, tag="rrow", name="rrow")
                    nc.vector.tensor_copy(rrow[0:1, :], av[64:65, :])
                    rcp = r_pool.tile([P, 512], F32, tag="rcp", name="rcp")
                    # custom-DVE op: base partition 0 only
                    nc.vector.reciprocal_approx_fast(rcp[0:1, :], rrow[0:1, :])
                    av_sb = r_pool.tile([P, 512], F32, tag="avsb", name="avsb")
                    nc.vector.tensor_copy(av_sb[0:64, :], av[0:64, :])
                    rbc = r_pool.tile([P, 512], F32, tag="rbc", name="rbc")
                    avsb2.append(av_sb)
                    rbc2.append(rbc)
                    hw = 512 // halves
                    for h in range(halves):
                        nc.gpsimd.partition_broadcast(
                            rbc[0:64, h * hw:(h + 1) * hw],
                            rcp[0:1, h * hw:(h + 1) * hw])
                hw = 512 // halves
                for h in range(halves):
                    for par in range(2):
                        # 64-channel DVE op writes the head's attnT quadrant
                        nc.vector.tensor_mul(
                            attnT[p][par * 64:(par + 1) * 64,
                                     qc * 512 + h * hw:qc * 512 + (h + 1) * hw],
                            avsb2[par][0:64, h * hw:(h + 1) * hw],
                            rbc2[par][0:64, h * hw:(h + 1) * hw])

            proj_osb = {}
            proj_ps = {}

            def proj_mms(nt, ci, ks, ke):
                c0, cw = ((0, 512), (512, 256))[ci]
                if ks == 0:
                    proj_ps[(nt, ci)] = g_ps.tile([P, 512], F32, tag="g",
                                                  name="gp")
                ps = proj_ps[(nt, ci)]
                for k in range(ks, ke):
                    nc.tensor.matmul(
                        ps[:, :cw],
                        attnT[k][:, nt * P:(nt + 1) * P],
                        wproj_sb[k][:, c0:c0 + cw],
                        start=(k == 0), stop=(k == KC - 1),
                    )

            def proj_fin(nt, ci):
                c0, cw = ((0, 512), (512, 256))[ci]
                ps = proj_ps.pop((nt, ci))
                if ci == 0:
                    proj_osb[nt] = o_pool.tile([P, C], F32, tag="o", name="osb")
                o_sb = proj_osb[nt]
                nc.vector.tensor_add(o_sb[:, c0:c0 + cw], ps[:, :cw],
                                     bias_sb[:, c0:c0 + cw])
                nc.sync.dma_start(out=out[nt * P:(nt + 1) * P, c0:c0 + cw],
                                  in_=o_sb[:, c0:c0 + cw])

            def emit_proj(nt, ci):
                proj_mms(nt, ci, 0, KC)
                proj_fin(nt, ci)

            def emit_scores_kt(p, qc, kt):
                ps = sc_ps.tile([P, N], F32, tag="sc", name="scps")
                nc.tensor.matmul(
                    ps[:, 0:512],
                    qk_sb[6 + p][0:64, kt * P:(kt + 1) * P],
                    qk_sb[p][0:64, qc * 512:(qc + 1) * 512],
                    start=True, stop=True, tile_position=(0, 0),
                )
                nc.tensor.matmul(
                    ps[:, 512:1024],
                    qk_sb[6 + p][64:128, kt * P:(kt + 1) * P],
                    qk_sb[p][64:128, qc * 512:(qc + 1) * 512],
                    start=True, stop=True, tile_position=(64, 0),
                )
                es = es_pool.tile([P, N], BF16, tag="es", name="es")
                nc.scalar.activation(es[:], ps[:], Exp, scale=SCALE)
                return es

            # ---------- PRE: v + qkT for pair 0, in DMA-arrival order ----
            for nt in range(NT):
                nc.vector.memset(v_sb[nt][:, :, 64:65], 1.0)
            for nt in range(4):
                emit_v(nt, 0)
            emit_qkT(0, 0)
            emit_qkT(6, 0)
            for nt in range(4):
                emit_v(nt, 1)
            for nt in range(4, NT):
                emit_v(nt, 0)
                emit_v(nt, 1)
            emit_qkT(0, 1)
            emit_qkT(6, 1)

            # ---------- attention with interleaved fillers ----------
            # iters 0..4 fillers: remaining qkT M-tiles (one pair ahead of
            # the scores that consume them); iters 6..9: proj of qc0 rows
            filler_map = {
                0: [(emit_qkT, (1, 0)), (emit_qkT, (1, 1)),
                    (emit_qkT, (7, 0)), (emit_qkT, (7, 1))],
                1: [(emit_qkT, (2, 0)), (emit_qkT, (2, 1)),
                    (emit_qkT, (8, 0)), (emit_qkT, (8, 1))],
                2: [(emit_qkT, (3, 0)), (emit_qkT, (3, 1)),
                    (emit_qkT, (9, 0)), (emit_qkT, (9, 1))],
                3: [(emit_qkT, (4, 0)), (emit_qkT, (4, 1)),
                    (emit_qkT, (10, 0)), (emit_qkT, (10, 1))],
                4: [(emit_qkT, (5, 0)), (emit_qkT, (5, 1)),
                    (emit_qkT, (11, 0)), (emit_qkT, (11, 1))],
                7: [(emit_proj, (0, 0)), (emit_proj, (0, 1))],
                8: [(emit_proj, (1, 0)), (emit_proj, (1, 1))],
                9: [(emit_proj, (2, 0)), (emit_proj, (2, 1))],
                10: [(emit_proj, (3, 0)), (emit_proj, (3, 1))],
            }
            pending = None
            self_av = None
            for it in range(12):
                qc, p = it // 6, it % 6
                fillers = list(filler_map.get(it, []))
                av_ps2 = av_alloc() if pending is not None else None
                # last iteration also drains its own AV (lag 2 behind the
                # exp pipeline) so the tail has no standalone AV pass
                if it == 11:
                    self_av = av_alloc()
                es_tiles = []
                for kt in range(NT):
                    es_tiles.append(emit_scores_kt(p, qc, kt))
                    if pending is not None:
                        # interleave previous pair's av accumulation between
                        # scores pairs (fills PE while exp runs, and lets the
                        # scores LDWEIGHTS background-load without row-group
                        # conflicts)
                        if it == 11:
                            # front-load: all es for the previous pair are
                            # ready, so drain its av at 2/kt and emit its
                            # norm mid-iteration where it hides under the
                            # remaining scores
                            if kt < 4:
                                av_mms(pending[0], pending[2], av_ps2, 2 * kt)
                                av_mms(pending[0], pending[2], av_ps2,
                                       2 * kt + 1)
                            elif kt == 4:
                                av_norm(pending[0], pending[1], av_ps2)
                        else:
                            av_mms(pending[0], pending[2], av_ps2, kt)
                    if self_av is not None and kt >= 2:
                        av_mms(p, es_tiles, self_av, kt - 2)
                    if kt % 2 == 1 and fillers:
                        fn, args = fillers.pop(0)
                        fn(*args)
                for fn, args in fillers:
                    fn(*args)
                if pending is not None and it != 11:
                    av_norm(pending[0], pending[1], av_ps2)
                pending = (p, qc, es_tiles)

            # ---------- tail: finish (5,1) av + norm, overlap with the ----
            # qc1 projections (k0-4 are independent of the last norm; the
            # norm runs split in q-halves so nt4/5's k5 unblocks first)
            es_tiles = pending[2]
            av_mms(5, es_tiles, self_av, 6)
            proj_mms(4, 0, 0, KC - 1)
            av_mms(5, es_tiles, self_av, 7)
            av_norm(5, 1, self_av, halves=2)
            proj_mms(4, 1, 0, KC - 1)
            proj_mms(5, 0, 0, KC - 1)
            proj_mms(5, 1, 0, KC - 1)
            proj_mms(4, 0, KC - 1, KC)
            proj_fin(4, 0)
            proj_mms(4, 1, KC - 1, KC)
            proj_fin(4, 1)
            proj_mms(5, 0, KC - 1, KC)
            proj_fin(5, 0)
            proj_mms(5, 1, KC - 1, KC)
            proj_fin(5, 1)
            for nt in range(6, NT):
                emit_proj(nt, 0)
                emit_proj(nt, 1)

    nc.finalize()
    return nc


_NC_CACHE = None


def _get_nc():
    global _NC_CACHE
    if _NC_CACHE is None:
        _NC_CACHE = build_nc()
    return _NC_CACHE


def prep_inputs(x, w_qkv, w_proj, b_proj):
    import ml_dtypes
    x = np.asarray(x, dtype=np.float32)
    w_qkv = np.asarray(w_qkv, dtype=np.float32)
    w_proj = np.asarray(w_proj, dtype=np.float32)
    b_proj = np.asarray(b_proj, dtype=np.float32)
    bf16 = ml_dtypes.bfloat16
    wqk = np.ascontiguousarray(w_qkv[:2 * C].T).astype(bf16)   # [768, 1536]
    wv = np.ascontiguousarray(w_qkv[2 * C:].T).astype(bf16)    # [768, 768]
    wp = np.ascontiguousarray(w_proj.T).astype(bf16)
    bias = np.ascontiguousarray(np.tile(b_proj[None, :], (P, 1)))  # [128, 768]
    in_maps = []
    for b in range(NCORES):
        in_maps.append({
            "xt": np.ascontiguousarray(x[b].T).astype(bf16),   # [768, 1024]
            "wqk": wqk, "wv": wv, "wproj": wp, "bias": bias,
        })
    return in_maps


def run(in_maps, **kw):
    nc = _get_nc()
    return run_bass_kernel_spmd(nc, in_maps, list(range(NCORES)), **kw)


def kernel(x, w_qkv, w_proj, b_proj):
    res = run(prep_inputs(x, w_qkv, w_proj, b_proj))
    return np.stack([res.results[b]["out"] for b in range(NCORES)], axis=0)


# revision 13
# speedup vs baseline: 1.0659x; 1.0138x over previous
"""Fused multi-head attention block (qkv proj + attention + out proj) for
Trainium2, batch-parallel across 8 NeuronCores.

Problem shapes (hardcoded): x [8, 1024, 768], w_qkv [2304, 768],
w_proj [768, 768], b_proj [768]; H=12 heads, HD=64.

Each core processes one batch element b. Layouts:
  qkT  [2C, N]  q,k transposed (bf16): head h -> tile h//2, parts (h%2)*64..
  v_sb [N, H, 65] v natural (bf16) + ones column per head (softmax sums)
  S.T = kT.T @ qT per head, K=64 row-tiled head pairs sharing the PE array
  P.T = exp(S.T/8) on ACT (bf16, max-subtraction skipped: scores ~N(0,1),
        max ~5.5, exp < 300 so fp32 PSUM never overflows)
  [av; sums].T = [V|1].T @ P.T (bf16, M=65), normalized by broadcasting
  1/sums across partitions; attn.T (bf16) -> proj + bias.

x / w_qkv / w_v stream in as bf16 (halves input DMA; rel err ~8e-3 vs
2e-2 budget). DMAs are ordered to match emission order so the PE never
waits at startup. Emission interleaves qkv/proj matmul groups into the
ACT-paced attention loop so the PE never idles; the final pair's AV is
interleaved into its own scores iteration and the tail projections are
split into k0-4 (independent of the last softmax norm) and k5
(dependent) so the tail has no serial PE stall.
"""
import numpy as np

import concourse.bacc as bacc
import concourse.tile as tile
from concourse import mybir
from concourse.bass_utils import run_bass_kernel_spmd

B, N, C = 8, 1024, 768
H, HD = 12, 64
P = 128
NCORES = 8
F32 = mybir.dt.float32
BF16 = mybir.dt.bfloat16
Exp = mybir.ActivationFunctionType.Exp
Cpy = mybir.ActivationFunctionType.Copy

KC = C // P          # 6 contraction chunks of 128 over C
NT = N // P          # 8 npos tiles of 128
QC = 2               # qpos halves of 512
NPAIR = H // 2       # 6 head pairs
SCALE = float(HD) ** -0.5


def build_nc():
    nc = bacc.Bacc("TRN2", target_bir_lowering=False, debug=False)

    xt = nc.declare_dram_parameter("xt", [C, N], BF16, isOutput=False)
    wqk = nc.declare_dram_parameter("wqk", [C, 2 * C], BF16, isOutput=False)
    wv = nc.declare_dram_parameter("wv", [C, C], BF16, isOutput=False)
    wproj = nc.declare_dram_parameter("wproj", [C, C], BF16, isOutput=False)
    bias = nc.declare_dram_parameter("bias", [P, C], F32, isOutput=False)
    out = nc.declare_dram_parameter("out", [N, C], F32, isOutput=True)

    with tile.TileContext(nc) as tc:
        with tc.tile_pool(name="qk", bufs=1) as qk_pool, \
             tc.tile_pool(name="vsb", bufs=1) as v_pool, \
             tc.tile_pool(name="attnT", bufs=1) as at_pool, \
             tc.tile_pool(name="p1in", bufs=1) as p1in, \
             tc.tile_pool(name="p3in", bufs=1) as p3in, \
             tc.tile_pool(name="es", bufs=16) as es_pool, \
             tc.tile_pool(name="rr", bufs=2) as r_pool, \
             tc.tile_pool(name="osb", bufs=3) as o_pool, \
             tc.tile_pool(name="scps", bufs=2, space="PSUM") as sc_ps, \
             tc.tile_pool(name="gps", bufs=4, space="PSUM") as g_ps:

            qk_sb = [qk_pool.tile([P, N], BF16, tag=f"qk{i}", name=f"qk{i}")
                     for i in range(12)]
            v_sb = [v_pool.tile([P, H, 65], BF16, tag=f"v{i}", name=f"v{i}")
                    for i in range(NT)]
            attnT = [at_pool.tile([P, N], BF16, tag=f"at{i}", name=f"at{i}")
                     for i in range(NPAIR)]
            xt_sb = [p1in.tile([P, N], BF16, tag=f"xt{k}", name=f"xts{k}")
                     for k in range(KC)]
            wqk_sb = [p1in.tile([P, 2 * C], BF16, tag=f"wqk{k}", name=f"wqks{k}")
                      for k in range(KC)]
            wv_sb = [p1in.tile([P, C], BF16, tag=f"wv{k}", name=f"wvs{k}")
                     for k in range(KC)]
            wproj_sb = [p3in.tile([P, C], BF16, tag=f"wp{k}", name=f"wps{k}")
                        for k in range(KC)]
            bias_sb = p3in.tile([P, C], F32, tag="bias", name="biassb")
            ones_sb = p3in.tile([P, 64], F32, tag="ones", name="ones1")

            # DMAs ordered to match PE emission, spread over THREE engine
            # queues -- DMA descriptor issue costs ~590ns of engine time,
            # so issue rate (not HBM bandwidth) gates arrival. sync: xt;
            # scalar: wv (then free for exp); gpsimd: wqk + proj weights
            # (gpsimd's first own work is the iter-1 softmax broadcast).
            for k in range(KC):
                nc.sync.dma_start(out=xt_sb[k][:, 0:512],
                                  in_=xt[k * P:(k + 1) * P, 0:512])
            for k in range(KC):
                nc.scalar.dma_start(out=wv_sb[k][:, 0:512],
                                    in_=wv[k * P:(k + 1) * P, 0:512])
            for k in range(KC):
                nc.gpsimd.dma_start(out=wqk_sb[k][:, 0:128],
                                    in_=wqk[k * P:(k + 1) * P, 0:128])
                nc.gpsimd.dma_start(out=wqk_sb[k][:, 768:896],
                                    in_=wqk[k * P:(k + 1) * P, 768:896])
            for k in range(KC):
                nc.scalar.dma_start(out=wv_sb[k][:, 512:768],
                                    in_=wv[k * P:(k + 1) * P, 512:768])
            for k in range(KC):
                nc.sync.dma_start(out=xt_sb[k][:, 512:1024],
                                  in_=xt[k * P:(k + 1) * P, 512:1024])
            for k in range(KC):
                nc.gpsimd.dma_start(out=wqk_sb[k][:, 128:768],
                                    in_=wqk[k * P:(k + 1) * P, 128:768])
                nc.gpsimd.dma_start(out=wqk_sb[k][:, 896:1536],
                                    in_=wqk[k * P:(k + 1) * P, 896:1536])
            for k in range(KC):
                nc.gpsimd.dma_start(out=wproj_sb[k][:],
                                    in_=wproj[k * P:(k + 1) * P, :])
            nc.gpsimd.dma_start(out=bias_sb[:], in_=bias[:, :])

            def emit_qkT(mt, nh):
                ps = g_ps.tile([P, 512], F32, tag="g", name="gq")
                for k in range(KC):
                    nc.tensor.matmul(
                        ps[:],
                        wqk_sb[k][:, mt * P:(mt + 1) * P],
                        xt_sb[k][:, nh * 512:(nh + 1) * 512],
                        start=(k == 0), stop=(k == KC - 1),
                    )
                nc.vector.tensor_copy(qk_sb[mt][:, nh * 512:(nh + 1) * 512], ps[:])

            def emit_v(nt, ci):
                c0, cw = ((0, 512), (512, 256))[ci]
                ps = g_ps.tile([P, 512], F32, tag="g", name="gv")
                for k in range(KC):
                    nc.tensor.matmul(
                        ps[:, :cw],
                        xt_sb[k][:, nt * P:(nt + 1) * P],
                        wv_sb[k][:, c0:c0 + cw],
                        start=(k == 0), stop=(k == KC - 1),
                    )
                psv = ps[:, :cw].rearrange("p (j q) -> p j q", q=64)
                nc.vector.tensor_copy(
                    v_sb[nt][:, c0 // 64:c0 // 64 + cw // 64, 0:64], psv[:])

            def av_alloc():
                return [g_ps.tile([P, 512], F32, tag="g", name="gav")
                        for _ in range(2)]

            def av_mms(p, es_tiles, av_ps2, kt):
                for par in range(2):
                    nc.tensor.matmul(
                        av_ps2[par][0:65, :],
                        v_sb[kt][:, 2 * p + par, :],
                        es_tiles[kt][:, par * 512:(par + 1) * 512],
                        start=(kt == 0), stop=(kt == NT - 1),
                    )

            def av_norm(p, qc, av_ps2, halves=1):
                # reciprocal runs on the [1,512] sums row BEFORE the
                # broadcast (bc(recip(x)) == recip(bc(x))), and the sums
                # row is read straight out of PSUM in parallel with the
                # data eviction -- short serial chain, less DVE work.
                # halves=2 splits bc+mul into q-halves so downstream proj
                # k5 consumers of the first half unblock earlier.
                avsb2, rbc2 = [], []
                for par in range(2):
                    av = av_ps2[par]
                    rrow = r_pool.tile([P, 512], F32, tag="rrow", name="rrow")
                    nc.vector.tensor_copy(rrow[0:1, :], av[64:65, :])
                    rcp = r_pool.tile([P, 512], F32, tag="rcp", name="rcp")
                    # custom-DVE op: base partition 0 only
                    nc.vector.reciprocal_approx_fast(rcp[0:1, :], rrow[0:1, :])
                    av_sb = r_pool.tile([P, 512], F32, tag="avsb", name="avsb")
                    nc.vector.tensor_copy(av_sb[0:64, :], av[0:64, :])
                    rbc = r_pool.tile([P, 512], F32, tag="rbc", name="rbc")
                    avsb2.append(av_sb)
                    rbc2.append(rbc)
                    hw = 512 // halves
                    for h in range(halves):
                        nc.gpsimd.partition_broadcast(
                            rbc[0:64, h * hw:(h + 1) * hw],
                            rcp[0:1, h * hw:(h + 1) * hw])
                hw = 512 // halves
                for h in range(halves):
                    for par in range(2):
                        # 64-channel DVE op writes the head's attnT quadrant
                        nc.vector.tensor_mul(
                            attnT[p][par * 64:(par + 1) * 64,
                                     qc * 512 + h * hw:qc * 512 + (h + 1) * hw],
                            avsb2[par][0:64, h * hw:(h + 1) * hw],
                            rbc2[par][0:64, h * hw:(h + 1) * hw])

            proj_osb = {}
            proj_ps = {}

            def proj_mms(nt, ci, ks, ke, ps=None):
                c0, cw = ((0, 512), (512, 256))[ci]
                if ks == 0:
                    proj_ps[(nt, ci)] = (ps if ps is not None else
                                         g_ps.tile([P, 512], F32, tag="g",
                                                   name="gp"))
                ps = proj_ps[(nt, ci)]
                for k in range(ks, ke):
                    nc.tensor.matmul(
                        ps[:, :cw],
                        attnT[k][:, nt * P:(nt + 1) * P],
                        wproj_sb[k][:, c0:c0 + cw],
                        start=(k == 0), stop=(k == KC - 1),
                    )

            def proj_fin(nt, ci, q=None):
                c0, cw = ((0, 512), (512, 256))[ci]
                ps = proj_ps.pop((nt, ci))
                if ci == 0:
                    proj_osb[nt] = o_pool.tile([P, C], F32, tag="o", name="osb")
                o_sb = proj_osb[nt]
                nc.vector.tensor_add(o_sb[:, c0:c0 + cw], ps[:, :cw],
                                     bias_sb[:, c0:c0 + cw])
                (q or nc.sync).dma_start(
                    out=out[nt * P:(nt + 1) * P, c0:c0 + cw],
                    in_=o_sb[:, c0:c0 + cw])

            def emit_proj(nt, ci):
                proj_mms(nt, ci, 0, KC)
                proj_fin(nt, ci)

            def emit_scores_kt(p, qc, kt):
                ps = sc_ps.tile([P, N], F32, tag="sc", name="scps")
                nc.tensor.matmul(
                    ps[:, 0:512],
                    qk_sb[6 + p][0:64, kt * P:(kt + 1) * P],
                    qk_sb[p][0:64, qc * 512:(qc + 1) * 512],
                    start=True, stop=True, tile_position=(0, 0),
                )
                nc.tensor.matmul(
                    ps[:, 512:1024],
                    qk_sb[6 + p][64:128, kt * P:(kt + 1) * P],
                    qk_sb[p][64:128, qc * 512:(qc + 1) * 512],
                    start=True, stop=True, tile_position=(64, 0),
                )
                es = es_pool.tile([P, N], BF16, tag="es", name="es")
                nc.scalar.activation(es[:], ps[:], Exp, scale=SCALE)
                return es

            # ---------- PRE: v + qkT for pair 0, in DMA-arrival order ----
            nc.vector.memset(ones_sb[0:1, :], 1.0)
            for nt in range(NT):
                nc.vector.memset(v_sb[nt][:, :, 64:65], 1.0)
            for nt in range(4):
                emit_v(nt, 0)
            emit_qkT(0, 0)
            emit_qkT(6, 0)
            for nt in range(4):
                emit_v(nt, 1)
            for nt in range(4, NT):
                emit_v(nt, 0)
                emit_v(nt, 1)
            emit_qkT(0, 1)
            emit_qkT(6, 1)

            # ---------- attention with interleaved fillers ----------
            # iters 0..4 fillers: remaining qkT M-tiles (one pair ahead of
            # the scores that consume them); iters 6..9: proj of qc0 rows
            filler_map = {
                0: [(emit_qkT, (1, 0)), (emit_qkT, (1, 1)),
                    (emit_qkT, (7, 0)), (emit_qkT, (7, 1))],
                1: [(emit_qkT, (2, 0)), (emit_qkT, (2, 1)),
                    (emit_qkT, (8, 0)), (emit_qkT, (8, 1))],
                2: [(emit_qkT, (3, 0)), (emit_qkT, (3, 1)),
                    (emit_qkT, (9, 0)), (emit_qkT, (9, 1))],
                3: [(emit_qkT, (4, 0)), (emit_qkT, (4, 1)),
                    (emit_qkT, (10, 0)), (emit_qkT, (10, 1))],
                4: [(emit_qkT, (5, 0)), (emit_qkT, (5, 1)),
                    (emit_qkT, (11, 0)), (emit_qkT, (11, 1))],
                7: [(emit_proj, (0, 0)), (emit_proj, (0, 1))],
                8: [(emit_proj, (1, 0)), (emit_proj, (1, 1))],
                9: [(emit_proj, (2, 0)), (emit_proj, (2, 1))],
                10: [(emit_proj, (3, 0)), (emit_proj, (3, 1))],
            }
            pending = None
            self_av = None
            for it in range(12):
                qc, p = it // 6, it % 6
                fillers = list(filler_map.get(it, []))
                av_ps2 = av_alloc() if pending is not None else None
                # last iteration also drains its own AV (lag 2 behind the
                # exp pipeline) so the tail has no standalone AV pass
                if it == 11:
                    self_av = av_alloc()
                es_tiles = []
                for kt in range(NT):
                    es_tiles.append(emit_scores_kt(p, qc, kt))
                    if pending is not None:
                        # interleave previous pair's av accumulation between
                        # scores pairs (fills PE while exp runs, and lets the
                        # scores LDWEIGHTS background-load without row-group
                        # conflicts)
                        if it == 11:
                            # front-load: all es for the previous pair are
                            # ready, so drain its av at 2/kt and emit its
                            # norm mid-iteration where it hides under the
                            # remaining scores
                            if kt < 4:
                                av_mms(pending[0], pending[2], av_ps2, 2 * kt)
                                av_mms(pending[0], pending[2], av_ps2,
                                       2 * kt + 1)
                            elif kt == 4:
                                av_norm(pending[0], pending[1], av_ps2)
                        else:
                            av_mms(pending[0], pending[2], av_ps2, kt)
                    if self_av is not None and kt >= 2:
                        av_mms(p, es_tiles, self_av, kt - 2)
                    if kt % 2 == 1 and fillers:
                        fn, args = fillers.pop(0)
                        fn(*args)
                for fn, args in fillers:
                    fn(*args)
                if pending is not None and it != 11:
                    av_norm(pending[0], pending[1], av_ps2)
                pending = (p, qc, es_tiles)

            # ---------- tail: finish (5,1) av + norm, overlap with the ----
            # qc1 projections. The last norm is spread across engines:
            # PSUM evictions on scalar (idle after its last exp),
            # reciprocals on vector, the partition-broadcast as a K=1
            # ones-matmul on the PE (into a free scores-PSUM tile), and
            # the muls split in q-halves so nt4/5's k5 unblocks first.
            # All k0-4 proj matmuls are independent of the norm and keep
            # the PE busy under it; nt6 borrows the other scores-PSUM
            # tile so only nt7 waits for free accumulator slots.
            es_tiles = pending[2]
            av_mms(5, es_tiles, self_av, 6)
            proj_mms(4, 0, 0, KC - 1)
            av_mms(5, es_tiles, self_av, 7)
            scbc = sc_ps.tile([P, N], F32, tag="sc", name="scbc")
            rcs, avsb = [], []
            for par in range(2):
                rrow = r_pool.tile([P, 512], F32, tag="rrow", name="rrow")
                nc.scalar.activation(rrow[0:1, :], self_av[par][64:65, :], Cpy)
                rcp = r_pool.tile([P, 512], F32, tag="rcp", name="rcp")
                nc.vector.reciprocal_approx_fast(rcp[0:1, :], rrow[0:1, :])
                rcs.append(rcp)
            for par in range(2):
                av_sb = r_pool.tile([P, 512], F32, tag="avsb", name="avsb")
                nc.scalar.activation(av_sb[0:64, :], self_av[par][0:64, :], Cpy)
                avsb.append(av_sb)
            proj_mms(4, 1, 0, KC - 1)
            scB = sc_ps.tile([P, N], F32, tag="sc", name="scB")
            proj_mms(6, 0, 0, KC - 1, ps=scB[:, 0:512])
            for par in range(2):
                nc.tensor.matmul(scbc[0:64, par * 512:(par + 1) * 512],
                                 ones_sb[0:1, :], rcs[par][0:1, :],
                                 start=True, stop=True)
            proj_mms(6, 1, 0, KC - 1, ps=scB[:, 512:1024])
            for h in range(2):
                for par in range(2):
                    nc.vector.tensor_mul(
                        attnT[5][par * 64:(par + 1) * 64,
                                 512 + h * 256:512 + (h + 1) * 256],
                        avsb[par][0:64, h * 256:(h + 1) * 256],
                        scbc[0:64, par * 512 + h * 256:par * 512 + (h + 1) * 256])
            proj_mms(5, 0, 0, KC - 1)
            proj_mms(5, 1, 0, KC - 1)
            for i, (nt, ci) in enumerate([(4, 0), (4, 1), (5, 0), (5, 1),
                                          (6, 0), (6, 1)]):
                proj_mms(nt, ci, KC - 1, KC)
                proj_fin(nt, ci, q=(nc.sync if i % 2 == 0 else nc.gpsimd))
            proj_mms(7, 0, 0, KC)
            proj_fin(7, 0, q=nc.sync)
            proj_mms(7, 1, 0, KC)
            proj_fin(7, 1, q=nc.gpsimd)

    nc.finalize()
    return nc


_NC_CACHE = None


def _get_nc():
    global _NC_CACHE
    if _NC_CACHE is None:
        _NC_CACHE = build_nc()
    return _NC_CACHE


def prep_inputs(x, w_qkv, w_proj, b_proj):
    import ml_dtypes
    x = np.asarray(x, dtype=np.float32)
    w_qkv = np.asarray(w_qkv, dtype=np.float32)
    w_proj = np.asarray(w_proj, dtype=np.float32)
    b_proj = np.asarray(b_proj, dtype=np.float32)
    bf16 = ml_dtypes.bfloat16
    wqk = np.ascontiguousarray(w_qkv[:2 * C].T).astype(bf16)   # [768, 1536]
    wv = np.ascontiguousarray(w_qkv[2 * C:].T).astype(bf16)    # [768, 768]
    wp = np.ascontiguousarray(w_proj.T).astype(bf16)
    bias = np.ascontiguousarray(np.tile(b_proj[None, :], (P, 1)))  # [128, 768]
    in_maps = []
    for b in range(NCORES):
        in_maps.append({
            "xt": np.ascontiguousarray(x[b].T).astype(bf16),   # [768, 1024]
            "wqk": wqk, "wv": wv, "wproj": wp, "bias": bias,
        })
    return in_maps


def run(in_maps, **kw):
    nc = _get_nc()
    return run_bass_kernel_spmd(nc, in_maps, list(range(NCORES)), **kw)


def kernel(x, w_qkv, w_proj, b_proj):
    res = run(prep_inputs(x, w_qkv, w_proj, b_proj))
    return np.stack([res.results[b]["out"] for b in range(NCORES)], axis=0)


# revision 23
# speedup vs baseline: 1.0729x; 1.0066x over previous
"""Fused multi-head attention block (qkv proj + attention + out proj) for
Trainium2, batch-parallel across 8 NeuronCores.

Problem shapes (hardcoded): x [8, 1024, 768], w_qkv [2304, 768],
w_proj [768, 768], b_proj [768]; H=12 heads, HD=64.

Each core processes one batch element b. Layouts:
  qkT  [2C, N]  q,k transposed (bf16): head h -> tile h//2, parts (h%2)*64..
  v_sb [N, H, 65] v natural (bf16) + ones column per head (softmax sums)
  S.T = kT.T @ qT per head, K=64 row-tiled head pairs sharing the PE array
  P.T = exp(S.T/8) on ACT (bf16, max-subtraction skipped: scores ~N(0,1),
        max ~5.5, exp < 300 so fp32 PSUM never overflows)
  [av; sums].T = [V|1].T @ P.T (bf16, M=65), normalized by broadcasting
  1/sums across partitions; attn.T (bf16) -> proj + bias.

x / w_qkv / w_v stream in as bf16 (halves input DMA; rel err ~8e-3 vs
2e-2 budget). DMAs are ordered to match emission order so the PE never
waits at startup. Emission interleaves qkv/proj matmul groups into the
ACT-paced attention loop so the PE never idles; the final pair's AV is
interleaved into its own scores iteration and the tail projections are
split into k0-4 (independent of the last softmax norm) and k5
(dependent) so the tail has no serial PE stall.
"""
import numpy as np

import concourse.bacc as bacc
import concourse.tile as tile
from concourse import mybir
from concourse.bass_utils import run_bass_kernel_spmd

B, N, C = 8, 1024, 768
H, HD = 12, 64
P = 128
NCORES = 8
F32 = mybir.dt.float32
BF16 = mybir.dt.bfloat16
Exp = mybir.ActivationFunctionType.Exp
Cpy = mybir.ActivationFunctionType.Copy

KC = C // P          # 6 contraction chunks of 128 over C
NT = N // P          # 8 npos tiles of 128
QC = 2               # qpos halves of 512
NPAIR = H // 2       # 6 head pairs
SCALE = float(HD) ** -0.5


def build_nc():
    nc = bacc.Bacc("TRN2", target_bir_lowering=False, debug=False)

    xt = nc.declare_dram_parameter("xt", [C, N], BF16, isOutput=False)
    wqk = nc.declare_dram_parameter("wqk", [C, 2 * C], BF16, isOutput=False)
    wv = nc.declare_dram_parameter("wv", [C, C], BF16, isOutput=False)
    wproj = nc.declare_dram_parameter("wproj", [C, C], BF16, isOutput=False)
    bias = nc.declare_dram_parameter("bias", [P, C], F32, isOutput=False)
    # bf16 output halves the end-of-kernel DMA drain; host upcasts
    out = nc.declare_dram_parameter("out", [N, C], BF16, isOutput=True)

    with tile.TileContext(nc) as tc:
        with tc.tile_pool(name="qk", bufs=1) as qk_pool, \
             tc.tile_pool(name="vsb", bufs=1) as v_pool, \
             tc.tile_pool(name="attnT", bufs=1) as at_pool, \
             tc.tile_pool(name="p1in", bufs=1) as p1in, \
             tc.tile_pool(name="p3in", bufs=1) as p3in, \
             tc.tile_pool(name="es", bufs=16) as es_pool, \
             tc.tile_pool(name="rr", bufs=2) as r_pool, \
             tc.tile_pool(name="osb", bufs=3) as o_pool, \
             tc.tile_pool(name="scps", bufs=2, space="PSUM") as sc_ps, \
             tc.tile_pool(name="gps", bufs=4, space="PSUM") as g_ps:

            qk_sb = [qk_pool.tile([P, N], BF16, tag=f"qk{i}", name=f"qk{i}")
                     for i in range(12)]
            v_sb = [v_pool.tile([P, H, 65], BF16, tag=f"v{i}", name=f"v{i}")
                    for i in range(NT)]
            attnT = [at_pool.tile([P, N], BF16, tag=f"at{i}", name=f"at{i}")
                     for i in range(NPAIR)]
            xt_sb = [p1in.tile([P, N], BF16, tag=f"xt{k}", name=f"xts{k}")
                     for k in range(KC)]
            wqk_sb = [p1in.tile([P, 2 * C], BF16, tag=f"wqk{k}", name=f"wqks{k}")
                      for k in range(KC)]
            wv_sb = [p1in.tile([P, C], BF16, tag=f"wv{k}", name=f"wvs{k}")
                     for k in range(KC)]
            wproj_sb = [p3in.tile([P, C], BF16, tag=f"wp{k}", name=f"wps{k}")
                        for k in range(KC)]
            bias_sb = p3in.tile([P, C], F32, tag="bias", name="biassb")
            ones_sb = p3in.tile([P, 64], F32, tag="ones", name="ones1")

            # DMAs ordered to match PE emission, spread over THREE engine
            # queues -- DMA descriptor issue costs ~590ns of engine time,
            # so issue rate (not HBM bandwidth) gates arrival. sync: xt;
            # scalar: wv (then free for exp); gpsimd: wqk + proj weights
            # (gpsimd's first own work is the iter-1 softmax broadcast).
            for k in range(KC):
                nc.sync.dma_start(out=xt_sb[k][:, 0:512],
                                  in_=xt[k * P:(k + 1) * P, 0:512])
            for k in range(KC):
                nc.scalar.dma_start(out=wv_sb[k][:, 0:512],
                                    in_=wv[k * P:(k + 1) * P, 0:512])
            # interleave q- and k-half slices per k so both first-pair qkT
            # stationaries land incrementally by ~7us (issue-rate bound)
            for k in range(KC):
                nc.gpsimd.dma_start(out=wqk_sb[k][:, 0:128],
                                    in_=wqk[k * P:(k + 1) * P, 0:128])
                nc.gpsimd.dma_start(out=wqk_sb[k][:, 768:896],
                                    in_=wqk[k * P:(k + 1) * P, 768:896])
            for k in range(KC):
                nc.scalar.dma_start(out=wv_sb[k][:, 512:768],
                                    in_=wv[k * P:(k + 1) * P, 512:768])
            for k in range(KC):
                nc.sync.dma_start(out=xt_sb[k][:, 512:1024],
                                  in_=xt[k * P:(k + 1) * P, 512:1024])
            for k in range(KC):
                nc.gpsimd.dma_start(out=wqk_sb[k][:, 128:768],
                                    in_=wqk[k * P:(k + 1) * P, 128:768])
                nc.gpsimd.dma_start(out=wqk_sb[k][:, 896:1536],
                                    in_=wqk[k * P:(k + 1) * P, 896:1536])
            for k in range(KC):
                nc.gpsimd.dma_start(out=wproj_sb[k][:],
                                    in_=wproj[k * P:(k + 1) * P, :])
            nc.gpsimd.dma_start(out=bias_sb[:], in_=bias[:, :])

            def emit_qkT(mt, nh):
                ps = g_ps.tile([P, 512], F32, tag="g", name="gq")
                for k in range(KC):
                    nc.tensor.matmul(
                        ps[:],
                        wqk_sb[k][:, mt * P:(mt + 1) * P],
                        xt_sb[k][:, nh * 512:(nh + 1) * 512],
                        start=(k == 0), stop=(k == KC - 1),
                    )
                nc.vector.tensor_copy(qk_sb[mt][:, nh * 512:(nh + 1) * 512], ps[:])

            def emit_v(nt, ci):
                c0, cw = ((0, 512), (512, 256))[ci]
                ps = g_ps.tile([P, 512], F32, tag="g", name="gv")
                for k in range(KC):
                    nc.tensor.matmul(
                        ps[:, :cw],
                        xt_sb[k][:, nt * P:(nt + 1) * P],
                        wv_sb[k][:, c0:c0 + cw],
                        start=(k == 0), stop=(k == KC - 1),
                    )
                psv = ps[:, :cw].rearrange("p (j q) -> p j q", q=64)
                nc.vector.tensor_copy(
                    v_sb[nt][:, c0 // 64:c0 // 64 + cw // 64, 0:64], psv[:])

            def av_alloc():
                return [g_ps.tile([P, 512], F32, tag="g", name="gav")
                        for _ in range(2)]

            def av_mms(p, es_tiles, av_ps2, kt):
                for par in range(2):
                    nc.tensor.matmul(
                        av_ps2[par][0:65, :],
                        v_sb[kt][:, 2 * p + par, :],
                        es_tiles[kt][:, par * 512:(par + 1) * 512],
                        start=(kt == 0), stop=(kt == NT - 1),
                    )

            def av_norm(p, qc, av_ps2, halves=1):
                # reciprocal runs on the [1,512] sums row BEFORE the
                # broadcast (bc(recip(x)) == recip(bc(x))), and the sums
                # row is read straight out of PSUM in parallel with the
                # data eviction -- short serial chain, less DVE work.
                # halves=2 splits bc+mul into q-halves so downstream proj
                # k5 consumers of the first half unblock earlier.
                avsb2, rbc2 = [], []
                for par in range(2):
                    av = av_ps2[par]
                    rrow = r_pool.tile([P, 512], F32, tag="rrow", name="rrow")
                    nc.vector.tensor_copy(rrow[0:1, :], av[64:65, :])
                    rcp = r_pool.tile([P, 512], F32, tag="rcp", name="rcp")
                    # custom-DVE op: base partition 0 only
                    nc.vector.reciprocal_approx_fast(rcp[0:1, :], rrow[0:1, :])
                    av_sb = r_pool.tile([P, 512], F32, tag="avsb", name="avsb")
                    nc.vector.tensor_copy(av_sb[0:64, :], av[0:64, :])
                    rbc = r_pool.tile([P, 512], F32, tag="rbc", name="rbc")
                    avsb2.append(av_sb)
                    rbc2.append(rbc)
                    hw = 512 // halves
                    for h in range(halves):
                        nc.gpsimd.partition_broadcast(
                            rbc[0:64, h * hw:(h + 1) * hw],
                            rcp[0:1, h * hw:(h + 1) * hw])
                hw = 512 // halves
                for h in range(halves):
                    for par in range(2):
                        # 64-channel DVE op writes the head's attnT quadrant
                        nc.vector.tensor_mul(
                            attnT[p][par * 64:(par + 1) * 64,
                                     qc * 512 + h * hw:qc * 512 + (h + 1) * hw],
                            avsb2[par][0:64, h * hw:(h + 1) * hw],
                            rbc2[par][0:64, h * hw:(h + 1) * hw])

            proj_osb = {}
            proj_ps = {}

            def proj_mms(nt, ci, ks, ke, ps=None):
                c0, cw = ((0, 512), (512, 256))[ci]
                if ks == 0:
                    proj_ps[(nt, ci)] = (ps if ps is not None else
                                         g_ps.tile([P, 512], F32, tag="g",
                                                   name="gp"))
                ps = proj_ps[(nt, ci)]
                for k in range(ks, ke):
                    nc.tensor.matmul(
                        ps[:, :cw],
                        attnT[k][:, nt * P:(nt + 1) * P],
                        wproj_sb[k][:, c0:c0 + cw],
                        start=(k == 0), stop=(k == KC - 1),
                    )

            def proj_fin(nt, ci, q=None):
                c0, cw = ((0, 512), (512, 256))[ci]
                ps = proj_ps.pop((nt, ci))
                if ci == 0:
                    proj_osb[nt] = o_pool.tile([P, C], BF16, tag="o",
                                               name="osb")
                o_sb = proj_osb[nt]
                nc.vector.tensor_add(o_sb[:, c0:c0 + cw], ps[:, :cw],
                                     bias_sb[:, c0:c0 + cw])
                (q or nc.sync).dma_start(
                    out=out[nt * P:(nt + 1) * P, c0:c0 + cw],
                    in_=o_sb[:, c0:c0 + cw])

            def emit_proj(nt, ci):
                proj_mms(nt, ci, 0, KC)
                proj_fin(nt, ci)

            def emit_scores_kt(p, qc, kt):
                ps = sc_ps.tile([P, N], F32, tag="sc", name="scps")
                nc.tensor.matmul(
                    ps[:, 0:512],
                    qk_sb[6 + p][0:64, kt * P:(kt + 1) * P],
                    qk_sb[p][0:64, qc * 512:(qc + 1) * 512],
                    start=True, stop=True, tile_position=(0, 0),
                )
                nc.tensor.matmul(
                    ps[:, 512:1024],
                    qk_sb[6 + p][64:128, kt * P:(kt + 1) * P],
                    qk_sb[p][64:128, qc * 512:(qc + 1) * 512],
                    start=True, stop=True, tile_position=(64, 0),
                )
                es = es_pool.tile([P, N], BF16, tag="es", name="es")
                nc.scalar.activation(es[:], ps[:], Exp, scale=SCALE)
                return es

            # ---------- PRE: v + qkT for pair 0, in DMA-arrival order ----
            nc.vector.memset(ones_sb[0:1, :], 1.0)
            for nt in range(NT):
                nc.vector.memset(v_sb[nt][:, :, 64:65], 1.0)
            for nt in range(4):
                emit_v(nt, 0)
            emit_qkT(0, 0)
            emit_qkT(6, 0)
            # warm the exp pipeline ~5us early: the first two score tiles
            # can run as soon as pair 0's qkT lands
            pre_es = [emit_scores_kt(0, 0, kt) for kt in range(2)]
            for nt in range(4):
                emit_v(nt, 1)
            for nt in range(4, NT):
                emit_v(nt, 0)
                emit_v(nt, 1)
            emit_qkT(0, 1)
            emit_qkT(6, 1)

            # ---------- attention with interleaved fillers ----------
            # iters 0..4 fillers: remaining qkT M-tiles (one pair ahead of
            # the scores that consume them); iters 6..9: proj of qc0 rows
            filler_map = {
                0: [(emit_qkT, (1, 0)), (emit_qkT, (1, 1)),
                    (emit_qkT, (7, 0)), (emit_qkT, (7, 1))],
                1: [(emit_qkT, (2, 0)), (emit_qkT, (2, 1)),
                    (emit_qkT, (8, 0)), (emit_qkT, (8, 1))],
                2: [(emit_qkT, (3, 0)), (emit_qkT, (3, 1)),
                    (emit_qkT, (9, 0)), (emit_qkT, (9, 1))],
                3: [(emit_qkT, (4, 0)), (emit_qkT, (4, 1)),
                    (emit_qkT, (10, 0)), (emit_qkT, (10, 1))],
                4: [(emit_qkT, (5, 0)), (emit_qkT, (5, 1)),
                    (emit_qkT, (11, 0)), (emit_qkT, (11, 1))],
                7: [(emit_proj, (0, 0)), (emit_proj, (0, 1))],
                8: [(emit_proj, (1, 0)), (emit_proj, (1, 1))],
                9: [(emit_proj, (2, 0)), (emit_proj, (2, 1))],
                10: [(emit_proj, (3, 0)), (emit_proj, (3, 1))],
            }
            pending = None
            self_av = None
            for it in range(12):
                qc, p = it // 6, it % 6
                fillers = list(filler_map.get(it, []))
                av_ps2 = av_alloc() if pending is not None else None
                # last iteration also drains its own AV (lag 2 behind the
                # exp pipeline) so the tail has no standalone AV pass
                if it == 11:
                    self_av = av_alloc()
                es_tiles = list(pre_es) if it == 0 else []
                for kt in range(len(es_tiles), NT):
                    es_tiles.append(emit_scores_kt(p, qc, kt))
                    if pending is not None:
                        # interleave previous pair's av accumulation between
                        # scores pairs (fills PE while exp runs, and lets the
                        # scores LDWEIGHTS background-load without row-group
                        # conflicts)
                        if it == 11:
                            # front-load: all es for the previous pair are
                            # ready, so drain its av at 2/kt and emit its
                            # norm mid-iteration where it hides under the
                            # remaining scores
                            if kt < 4:
                                av_mms(pending[0], pending[2], av_ps2, 2 * kt)
                                av_mms(pending[0], pending[2], av_ps2,
                                       2 * kt + 1)
                            elif kt == 4:
                                av_norm(pending[0], pending[1], av_ps2)
                        else:
                            av_mms(pending[0], pending[2], av_ps2, kt)
                    if self_av is not None and kt >= 2:
                        av_mms(p, es_tiles, self_av, kt - 2)
                    if kt % 2 == 1 and fillers:
                        fn, args = fillers.pop(0)
                        fn(*args)
                for fn, args in fillers:
                    fn(*args)
                if pending is not None and it != 11:
                    av_norm(pending[0], pending[1], av_ps2)
                pending = (p, qc, es_tiles)

            # ---------- tail: finish (5,1) av + norm, overlap with the ----
            # qc1 projections. The last norm is spread across engines:
            # PSUM evictions on scalar (idle after its last exp),
            # reciprocals on vector, the partition-broadcast as a K=1
            # ones-matmul on the PE (into a free scores-PSUM tile), and
            # the muls split in q-halves so nt4/5's k5 unblocks first.
            # All k0-4 proj matmuls are independent of the norm and keep
            # the PE busy under it; nt6 borrows the other scores-PSUM
            # tile so only nt7 waits for free accumulator slots.
            es_tiles = pending[2]
            av_mms(5, es_tiles, self_av, 6)
            proj_mms(4, 0, 0, KC - 1)
            av_mms(5, es_tiles, self_av, 7)
            scbc = sc_ps.tile([P, N], F32, tag="sc", name="scbc")
            rcs, avsb = [], []
            for par in range(2):
                rrow = r_pool.tile([P, 512], F32, tag="rrow", name="rrow")
                nc.scalar.activation(rrow[0:1, :], self_av[par][64:65, :], Cpy)
                rcp = r_pool.tile([P, 512], F32, tag="rcp", name="rcp")
                nc.vector.reciprocal_approx_fast(rcp[0:1, :], rrow[0:1, :])
                rcs.append(rcp)
            for par in range(2):
                av_sb = r_pool.tile([P, 512], F32, tag="avsb", name="avsb")
                nc.scalar.activation(av_sb[0:64, :], self_av[par][0:64, :], Cpy)
                avsb.append(av_sb)
            proj_mms(4, 1, 0, KC - 1)
            scB = sc_ps.tile([P, N], F32, tag="sc", name="scB")
            proj_mms(6, 0, 0, KC - 1, ps=scB[:, 0:512])
            for par in range(2):
                nc.tensor.matmul(scbc[0:64, par * 512:(par + 1) * 512],
                                 ones_sb[0:1, :], rcs[par][0:1, :],
                                 start=True, stop=True)
            proj_mms(6, 1, 0, KC - 1, ps=scB[:, 512:1024])
            for h in range(2):
                for par in range(2):
                    nc.vector.tensor_mul(
                        attnT[5][par * 64:(par + 1) * 64,
                                 512 + h * 256:512 + (h + 1) * 256],
                        avsb[par][0:64, h * 256:(h + 1) * 256],
                        scbc[0:64, par * 512 + h * 256:par * 512 + (h + 1) * 256])
            proj_mms(5, 0, 0, KC - 1)
            proj_mms(5, 1, 0, KC - 1)
            for i, (nt, ci) in enumerate([(4, 0), (4, 1), (5, 0), (5, 1),
                                          (6, 0), (6, 1)]):
                proj_mms(nt, ci, KC - 1, KC)
                proj_fin(nt, ci, q=(nc.sync if i % 2 == 0 else nc.gpsimd))
            proj_mms(7, 0, 0, KC)
            proj_fin(7, 0, q=nc.sync)
            proj_mms(7, 1, 0, KC)
            proj_fin(7, 1, q=nc.gpsimd)

    nc.finalize()
    return nc


_NC_CACHE = None


def _get_nc():
    global _NC_CACHE
    if _NC_CACHE is None:
        _NC_CACHE = build_nc()
    return _NC_CACHE


def prep_inputs(x, w_qkv, w_proj, b_proj):
    import ml_dtypes
    x = np.asarray(x, dtype=np.float32)
    w_qkv = np.asarray(w_qkv, dtype=np.float32)
    w_proj = np.asarray(w_proj, dtype=np.float32)
    b_proj = np.asarray(b_proj, dtype=np.float32)
    bf16 = ml_dtypes.bfloat16
    wqk = np.ascontiguousarray(w_qkv[:2 * C].T).astype(bf16)   # [768, 1536]
    wv = np.ascontiguousarray(w_qkv[2 * C:].T).astype(bf16)    # [768, 768]
    wp = np.ascontiguousarray(w_proj.T).astype(bf16)
    bias = np.ascontiguousarray(np.tile(b_proj[None, :], (P, 1)))  # [128, 768]
    in_maps = []
    for b in range(NCORES):
        in_maps.append({
            "xt": np.ascontiguousarray(x[b].T).astype(bf16),   # [768, 1024]
            "wqk": wqk, "wv": wv, "wproj": wp, "bias": bias,
        })
    return in_maps


def run(in_maps, **kw):
    nc = _get_nc()
    return run_bass_kernel_spmd(nc, in_maps, list(range(NCORES)), **kw)


def kernel(x, w_qkv, w_proj, b_proj):
    res = run(prep_inputs(x, w_qkv, w_proj, b_proj))
    return np.stack([np.asarray(res.results[b]["out"], dtype=np.float32)
                     for b in range(NCORES)], axis=0)
